# revision 27
# baseline (speedup 1.0000x reference)
"""Trainium2 Bass kernel: single-step attentive GRU decoder.

Model (B=64, L=2048, H=512, V=32000):
  x = emb[input_seq]; GRU cell -> h_new; dot-attention over encoder_outputs;
  concat -> linear -> tanh; 32000-vocab projection.

Distribution over 8 NeuronCores:
  - Batch-parallel attention: core i owns batch rows [8i, 8i+8) and its
    encoder_outputs shard (fp16, 16.8 MiB), read from HBM exactly once into
    SBUF-resident tiles.
  - Scores: fused multiply-reduce chunks split across DVE (scalar_tensor_tensor
    + accum) and GPSIMD-mul + ScalarE-reduce; softmax batched across all 8
    rows (GPSIMD partition_all_reduce for the cross-partition max/sum).
  - Context via TensorE matmuls on the same natural-layout tiles.
  - concat activations all-gathered (tiny), then the vocab projection is
    column-sharded: core i computes logits for vocab [4000i, 4000i+4000).

Host side shards/casts inputs (fp16), gathers embedding rows, transposes the
small weight matrices, and reassembles the full outputs.
"""

import numpy as np

B, L, H, V = 64, 2048, 512, 32000
NCORES = 8
BS = B // NCORES          # 8 batch rows per core
VS = V // NCORES          # 4000 vocab columns per core
VSP = 4096                # padded vocab slice
LC = L // 128             # 16 l-chunks of 128

_CACHE = {}


def _build_nc():
    """Build the (single, SPMD) Bass graph run on each of the 8 cores."""
    import os
    from contextlib import ExitStack

    import concourse.bass as bass
    from concourse import bacc, bass_isa, mybir, tile
    from concourse.masks import make_identity

    GPS_CH = int(os.environ.get("K_GPS", "24"))    # gpsimd chunks (group 0 only)
    ACT_CH = int(os.environ.get("K_ACT", "40"))    # DVE-mul + ACT-reduce chunks

    F16 = mybir.dt.float16
    F32 = mybir.dt.float32
    AF = mybir.ActivationFunctionType
    OP = mybir.AluOpType
    RED = bass_isa.ReduceOp

    nc = bacc.Bacc(None, target_bir_lowering=False, num_devices=NCORES)

    enc_d = nc.declare_dram_parameter("enc", [BS, L, H], F16, isOutput=False)
    xT_d = nc.declare_dram_parameter("xT", [H, BS], F16, isOutput=False)
    hT_d = nc.declare_dram_parameter("hT", [H, BS], F16, isOutput=False)
    hprev_d = nc.declare_dram_parameter("h_prev", [BS, H], F32, isOutput=False)
    Wi_d = nc.declare_dram_parameter("Wi", [H + 1, 3 * H], F16, isOutput=False)
    Wh_d = nc.declare_dram_parameter("Wh", [H + 1, 3 * H], F16, isOutput=False)
    Wc_d = nc.declare_dram_parameter("Wc", [2 * H + 1, H], F16, isOutput=False)
    Wo_d = nc.declare_dram_parameter("Wo", [H + 1, VSP], F16, isOutput=False)

    out_logits = nc.declare_dram_parameter("out_logits", [B, VSP], F32, isOutput=True)
    out_h = nc.declare_dram_parameter("out_h", [BS, H], F32, isOutput=True)
    out_attn = nc.declare_dram_parameter("out_attn", [BS, L], F32, isOutput=True)

    h16d = nc.dram_tensor("h16d", [BS, H], F16)
    cc_in = nc.dram_tensor("cc_in", [BS, H], F32)
    cc_out = nc.dram_tensor("cc_out", [B, H], F32, addr_space="Shared")

    with tile.TileContext(nc) as tc, ExitStack() as ctx:
        consts = ctx.enter_context(tc.tile_pool(name="consts", bufs=1))
        work = ctx.enter_context(tc.tile_pool(name="work", bufs=2))
        psum = ctx.enter_context(tc.tile_pool(name="psum", bufs=2, space="PSUM"))

        # ---- constants ----
        ident = consts.tile([64, 64], F32)
        make_identity(nc, ident)
        ones8 = consts.tile([1, BS], F16)
        nc.vector.memset(ones8, 1.0)
        ones64 = consts.tile([1, B], F16)
        nc.vector.memset(ones64, 1.0)
        WcK = consts.tile([128, 8, H], F16)
        Wcb = consts.tile([1, H], F16)

        # permanent small state
        co_all = consts.tile([BS, H], F32, tag="co_all")
        h_new = consts.tile([BS, H], F32, tag="h_new")
        h16 = consts.tile([BS, H], F16, tag="h16")
        catT = consts.tile([128, 8, BS], F16)
        hb = consts.tile([128, BS, H], F16)

        # ---- GRU phase (weights in a pool released afterwards) ----
        with tc.tile_pool(name="gru", bufs=1) as grup:
            WiK = grup.tile([128, 4, 3 * H], F16)
            nc.sync.dma_start(
                out=WiK, in_=Wi_d[0:H].rearrange("(c p) n -> p c n", p=128)
            )
            WhK = grup.tile([128, 4, 3 * H], F16)
            nc.sync.dma_start(
                out=WhK, in_=Wh_d[0:H].rearrange("(c p) n -> p c n", p=128)
            )
            Wib = grup.tile([1, 3 * H], F16)
            nc.sync.dma_start(out=Wib, in_=Wi_d[H : H + 1])
            Whbn = grup.tile([1, H], F16)
            nc.sync.dma_start(out=Whbn, in_=Wh_d[H : H + 1, 2 * H : 3 * H])
            xTs = grup.tile([128, 4, BS], F16)
            nc.sync.dma_start(out=xTs, in_=xT_d.rearrange("(c p) m -> p c m", p=128))
            hTs = grup.tile([128, 4, BS], F16)
            nc.sync.dma_start(out=hTs, in_=hT_d.rearrange("(c p) m -> p c m", p=128))
            hpv = grup.tile([BS, H], F32)
            nc.sync.dma_start(out=hpv, in_=hprev_d[:, :])

            # gates r, z: gi+gh accumulated in one PSUM (host folded b_hh's
            # r/z parts into Wi's bias row)
            rz = []
            for g in range(2):
                ps = psum.tile([BS, H], F32, tag="a")
                for c in range(4):
                    nc.tensor.matmul(
                        ps, xTs[:, c, :], WiK[:, c, g * H : (g + 1) * H],
                        start=(c == 0), stop=False,
                    )
                for c in range(4):
                    nc.tensor.matmul(
                        ps, hTs[:, c, :], WhK[:, c, g * H : (g + 1) * H],
                        start=False, stop=False,
                    )
                nc.tensor.matmul(ps, ones8, Wib[:, g * H : (g + 1) * H],
                                 start=False, stop=True)
                act = grup.tile([BS, H], F32, tag=f"gate{g}")
                nc.scalar.activation(act, ps, AF.Sigmoid)
                rz.append(act)
            r_sb, z_sb = rz

            gi_n = psum.tile([BS, H], F32, tag="a")
            for c in range(4):
                nc.tensor.matmul(gi_n, xTs[:, c, :], WiK[:, c, 2 * H : 3 * H],
                                 start=(c == 0), stop=False)
            nc.tensor.matmul(gi_n, ones8, Wib[:, 2 * H : 3 * H],
                             start=False, stop=True)
            gh_n = psum.tile([BS, H], F32, tag="b")
            for c in range(4):
                nc.tensor.matmul(gh_n, hTs[:, c, :], WhK[:, c, 2 * H : 3 * H],
                                 start=(c == 0), stop=False)
            nc.tensor.matmul(gh_n, ones8, Whbn, start=False, stop=True)

            rhn = grup.tile([BS, H], F32, tag="rhn")
            nc.vector.tensor_mul(rhn, r_sb, gh_n)
            tn = grup.tile([BS, H], F32, tag="tn")
            nc.vector.tensor_add(tn, gi_n, rhn)
            n_sb = grup.tile([BS, H], F32, tag="n_sb")
            nc.scalar.activation(n_sb, tn, AF.Tanh)
            dd = grup.tile([BS, H], F32, tag="dd")
            nc.vector.tensor_sub(dd, hpv, n_sb)
            zd = grup.tile([BS, H], F32, tag="zd")
            nc.vector.tensor_mul(zd, z_sb, dd)
            nc.vector.tensor_add(h_new, zd, n_sb)
            nc.sync.dma_start(out=out_h[:, :], in_=h_new)
            nc.scalar.copy(h16, h_new)

        # h_new^T (fp16 for the concat matmul, fp32 for the broadcast)
        for c in range(4):
            tp = psum.tile([128, BS], F32, tag="c")
            nc.tensor.transpose(tp, h_new[:, c * 128 : (c + 1) * 128],
                                ident[0:BS, 0:BS])
            nc.scalar.copy(catT[:, c, :], tp)

        # broadcast each batch row of h_new across all 128 partitions
        # (bounce through a DRAM fp16 scratch; partition-stride-0 DMA reads)
        nc.sync.dma_start(out=h16d[:, :], in_=h16)
        _h16d_ap = h16d[:, :]
        nc.gpsimd.dma_start(
            out=hb,
            in_=bass.AP(
                tensor=_h16d_ap.tensor, offset=_h16d_ap.offset,
                ap=[[0, 128], [H, BS], [1, H]],
            ),
        )

        # ---- attention, processed in two groups of 4 batch rows ----
        GS = 4
        attp = ctx.enter_context(tc.tile_pool(name="attp", bufs=1))
        et = attp.tile([128, BS, LC, H], F16)
        for b in range(BS):
            # partition p holds rows l in [16p, 16p+16) -> fully contiguous
            # 16 KiB per-partition DMA reads
            nc.sync.dma_start(
                out=et[:, b],
                in_=enc_d[b].rearrange("(p c) h -> p c h", p=128),
            )
        nc.sync.dma_start(
            out=WcK, in_=Wc_d[0 : 2 * H].rearrange("(c p) n -> p c n", p=128)
        )
        nc.sync.dma_start(out=Wcb, in_=Wc_d[2 * H : 2 * H + 1])
        WoK = attp.tile([128, 4, VSP], F16)
        nc.sync.dma_start(
            out=WoK, in_=Wo_d[0:H].rearrange("(c p) n -> p c n", p=128)
        )
        wobs = []
        for hh in range(2):
            wob = consts.tile([1, VSP // 2], F16, tag=f"wob{hh}")
            nc.sync.dma_start(
                out=wob, in_=Wo_d[H : H + 1, hh * 2048 : (hh + 1) * 2048]
            )
            wobs.append(wob)

        scr = consts.tile([128, BS, LC], F32)
        attn_l = consts.tile([128, BS, LC], F16)
        m1 = consts.tile([128, BS], F32)
        m1n = consts.tile([128, BS], F32)
        z1 = consts.tile([128, BS], F32)
        mg = consts.tile([128, BS], F32)
        zg = consts.tile([128, BS], F32)
        lnz = consts.tile([128, BS], F32)
        negc = consts.tile([128, BS], F32)

        for g in range(2):
            rows = range(g * GS, (g + 1) * GS)
            gsl = slice(g * GS, (g + 1) * GS)
            # -- phase A: score chunks. GPSIMD gets a front-loaded share in
            # group 0 only, so its queue is free for the softmax
            # partition-reduces and the collectives later; the rest is split
            # between DVE fused multiply-accumulate and DVE-mul + ACT-reduce.
            nact = ACT_CH // 2
            chunks = [(b, c) for b in rows for c in range(LC)]
            for j, (b, c) in enumerate(chunks):
                if j < GPS_CH:
                    prod2 = work.tile([128, H], F16, tag="prod2")
                    nc.gpsimd.tensor_mul(prod2, et[:, b, c, :], hb[:, b, :])
                    prod3 = consts.tile([128, H], F16, tag="prod3")
                    nc.scalar.activation(
                        prod3, prod2, AF.Identity,
                        accum_out=scr[:, b, c : c + 1],
                    )
                elif (j % len(chunks)) % 3 == 2 and nact > 0:
                    nact -= 1
                    prod2 = work.tile([128, H], F16, tag="prod2")
                    nc.vector.tensor_mul(prod2, et[:, b, c, :], hb[:, b, :])
                    prod3 = consts.tile([128, H], F16, tag="prod3")
                    nc.scalar.activation(
                        prod3, prod2, AF.Identity,
                        accum_out=scr[:, b, c : c + 1],
                    )
                else:
                    prod = consts.tile([128, H], F16, tag="prod")
                    nc.vector.scalar_tensor_tensor(
                        out=prod, in0=et[:, b, c, :], scalar=1.0,
                        in1=hb[:, b, :],
                        op0=OP.mult, op1=OP.mult,
                        accum_out=scr[:, b, c : c + 1],
                    )

            # -- phase B: softmax batched across the group's rows --
            # per-partition max over l-chunks (one 3D reduce), then global
            # max/sum across partitions; attn = exp(s - Mg - ln Z)
            nc.vector.tensor_reduce(
                m1[:, gsl], scr[:, gsl.start : gsl.stop],
                axis=mybir.AxisListType.X, op=OP.max,
            )
            nc.gpsimd.partition_all_reduce(
                mg[:, gsl], m1[:, gsl], channels=128, reduce_op=RED.max
            )
            nc.vector.tensor_scalar_mul(m1n[:, gsl], mg[:, gsl], -1.0)
            for b in rows:
                ex = work.tile([128, LC], F32, tag="ex")
                nc.scalar.activation(
                    ex, scr[:, b], AF.Exp, bias=m1n[:, b : b + 1],
                    accum_out=z1[:, b : b + 1],
                )
            nc.gpsimd.partition_all_reduce(
                zg[:, gsl], z1[:, gsl], channels=128, reduce_op=RED.add
            )
            nc.scalar.activation(lnz[:, gsl], zg[:, gsl], AF.Ln)
            # negc = -(Mg + lnZ) = -Mg - lnZ
            nc.vector.scalar_tensor_tensor(
                out=negc[:, gsl], in0=lnz[:, gsl], scalar=-1.0, in1=m1n[:, gsl],
                op0=OP.mult, op1=OP.add,
            )

            # -- phase C: attention weights out + context matmuls --
            ctx_g = consts.tile([GS, H], F32, tag="ctx_g")
            for b in rows:
                nc.scalar.activation(
                    attn_l[:, b], scr[:, b], AF.Exp, bias=negc[:, b : b + 1]
                )
                af = work.tile([128, LC], F32, tag="af")
                nc.scalar.activation(af, scr[:, b], AF.Exp,
                                     bias=negc[:, b : b + 1])
                nc.sync.dma_start(
                    out=out_attn[b].rearrange("(p c) -> p c", p=128), in_=af
                )
                ctxp = psum.tile([1, H], F32, tag="b")
                for c in range(LC):
                    nc.tensor.matmul(
                        ctxp, attn_l[:, b, c : c + 1], et[:, b, c, :],
                        start=(c == 0), stop=(c == LC - 1),
                    )
                ctx_b = consts.tile([1, H], F32, tag="ctx_b")
                nc.scalar.copy(ctx_b, ctxp)
                nc.sync.dma_start(out=ctx_g[b - g * GS : b - g * GS + 1, :],
                                  in_=ctx_b)

            # context^T into the concat stationary operand (columns gsl)
            for c in range(4):
                tp2 = psum.tile([128, GS], F32, tag="c")
                nc.tensor.transpose(tp2, ctx_g[:, c * 128 : (c + 1) * 128],
                                    ident[0:GS, 0:GS])
                nc.scalar.copy(catT[:, 4 + c, gsl], tp2)

            # -- concat -> linear -> tanh for this group's rows --
            gcp = psum.tile([GS, H], F32, tag="c")
            for j in range(8):
                nc.tensor.matmul(gcp, catT[:, j, gsl], WcK[:, j, :],
                                 start=(j == 0), stop=False)
            nc.tensor.matmul(gcp, ones8[:, 0:GS], Wcb, start=False, stop=True)
            co_g = consts.tile([GS, H], F32, tag="co_g")
            nc.scalar.activation(co_g, gcp, AF.Tanh)
            nc.sync.dma_start(
                out=co_all[g * GS : (g + 1) * GS, :], in_=co_g
            )

            if g == 1:
                # -- all-gather the concat activations (one collective),
                # then the vocab-sharded projection over all 64 rows --
                nc.sync.dma_start(out=cc_in[:, :], in_=co_all)
                nc.gpsimd.collective_compute(
                    "AllGather",
                    OP.bypass,
                    replica_groups=[list(range(NCORES))],
                    ins=[cc_in.ap().opt()],
                    outs=[cc_out.ap().opt()],
                )
                ca = consts.tile([B, H], F32, tag="ca")
                nc.sync.dma_start(out=ca, in_=cc_out[:, :])
                caT = consts.tile([128, 4, B], F16, tag="caT")
                for c in range(4):
                    tp3 = psum.tile([128, B], F32, tag="c")
                    nc.tensor.transpose(tp3, ca[:, c * 128 : (c + 1) * 128],
                                        ident[0:B, 0:B])
                    nc.scalar.copy(caT[:, c, :], tp3)

                for nch in range(8):
                    lp = psum.tile([B, 512], F32, tag="a")
                    for c in range(4):
                        nc.tensor.matmul(
                            lp, caT[:, c, :],
                            WoK[:, c, nch * 512 : (nch + 1) * 512],
                            start=(c == 0), stop=False,
                        )
                    nc.tensor.matmul(
                        lp, ones64,
                        wobs[nch // 4][:, (nch % 4) * 512 : (nch % 4 + 1) * 512],
                        start=False, stop=True,
                    )
                    lsb = work.tile([B, 512], F16, tag="lsb")
                    nc.scalar.copy(lsb, lp)
                    # SWDGE cast back to f32 on the way out
                    nc.gpsimd.dma_start(
                        out=out_logits[:, nch * 512 : (nch + 1) * 512],
                        in_=lsb,
                    )

    nc.compile()
    return nc


def make_in_maps(inputs):
    """Shard + cast the full inputs into per-core input maps."""
    f16 = np.float16
    f32 = np.float32
    emb = np.asarray(inputs["emb_table"], f32)
    seq = np.asarray(inputs["input_seq"]).astype(np.int64)
    h0 = np.asarray(inputs["last_hidden"], f32)[0]          # (64, 512)
    enc = np.asarray(inputs["encoder_outputs"], f32)        # (64, 2048, 512)
    W_ih = np.asarray(inputs["W_ih"], f32)
    W_hh = np.asarray(inputs["W_hh"], f32)
    b_ih = np.asarray(inputs["b_ih"], f32)
    b_hh = np.asarray(inputs["b_hh"], f32)
    W_concat = np.asarray(inputs["W_concat"], f32)
    b_concat = np.asarray(inputs["b_concat"], f32)
    W_out = np.asarray(inputs["W_out"], f32)
    b_out = np.asarray(inputs["b_out"], f32)

    x = emb[seq]                                            # (64, 512)
    enc16 = enc.astype(f16)
    # fold b_hh's r/z parts into Wi's bias row (kernel adds Wi's bias once
    # for r/z); Wh's bias row only contributes its n part
    bi = b_ih.copy()
    bi[0 : 2 * H] += b_hh[0 : 2 * H]
    Wi = np.ascontiguousarray(np.vstack([W_ih.T, bi[None]]).astype(f16))
    Wh = np.ascontiguousarray(np.vstack([W_hh.T, b_hh[None]]).astype(f16))
    Wc = np.ascontiguousarray(np.vstack([W_concat.T, b_concat[None]]).astype(f16))

    in_maps = []
    for i in range(NCORES):
        bs = slice(i * BS, (i + 1) * BS)
        vs = slice(i * VS, (i + 1) * VS)
        Wo = np.zeros((H + 1, VSP), f16)
        Wo[0:H, 0:VS] = W_out[vs].T
        Wo[H, 0:VS] = b_out[vs]
        in_maps.append(
            {
                "enc": np.ascontiguousarray(enc16[bs]),
                "xT": np.ascontiguousarray(x[bs].T.astype(f16)),
                "hT": np.ascontiguousarray(h0[bs].T.astype(f16)),
                "h_prev": np.ascontiguousarray(h0[bs]),
                "Wi": Wi,
                "Wh": Wh,
                "Wc": Wc,
                "Wo": Wo,
            }
        )
    return in_maps


def assemble_outputs(results, inputs):
    f32 = np.float32
    output = np.concatenate(
        [np.asarray(results[i]["out_logits"], f32)[:, :VS] for i in range(NCORES)],
        axis=1,
    )
    h_new = np.concatenate(
        [np.asarray(results[i]["out_h"], f32) for i in range(NCORES)], axis=0
    )[None]
    attn = np.concatenate(
        [np.asarray(results[i]["out_attn"], f32).reshape(BS, L) for i in range(NCORES)],
        axis=0,
    )[:, None, :]
    feed = np.asarray(inputs["input_feeding_prev_time_step"], f32)
    return output, h_new, attn, feed


def get_nc():
    if "nc" not in _CACHE:
        _CACHE["nc"] = _build_nc()
    return _CACHE["nc"]


def kernel(**inputs):
    from concourse.bass_utils import run_bass_kernel_spmd

    nc = get_nc()
    in_maps = make_in_maps(inputs)
    res = run_bass_kernel_spmd(nc, in_maps, core_ids=list(range(NCORES))).results
    return assemble_outputs(res, inputs)


# revision 28
# speedup vs baseline: 1.2988x; 1.2988x over previous
"""Trainium2 Bass kernel: single-step attentive GRU decoder.

Model (B=64, L=2048, H=512, V=32000):
  x = emb[input_seq]; GRU cell -> h_new; dot-attention over encoder_outputs;
  concat -> linear -> tanh; 32000-vocab projection.

Distribution over 8 NeuronCores:
  - Batch-parallel attention: core i owns batch rows [8i, 8i+8) and its
    encoder_outputs shard (fp16, 16.8 MiB), read from HBM exactly once into
    SBUF-resident tiles.
  - Scores: fused multiply-reduce chunks split across DVE (scalar_tensor_tensor
    + accum) and GPSIMD-mul + ScalarE-reduce; softmax batched across all 8
    rows (GPSIMD partition_all_reduce for the cross-partition max/sum).
  - Context via TensorE matmuls on the same natural-layout tiles.
  - concat activations all-gathered (tiny), then the vocab projection is
    column-sharded: core i computes logits for vocab [4000i, 4000i+4000).

Host side shards/casts inputs (fp16), gathers embedding rows, transposes the
small weight matrices, and reassembles the full outputs.
"""

import numpy as np

B, L, H, V = 64, 2048, 512, 32000
NCORES = 8
BS = B // NCORES          # 8 batch rows per core
VS = V // NCORES          # 4000 vocab columns per core
VSP = 4096                # padded vocab slice
LC = L // 128             # 16 l-chunks of 128

_CACHE = {}


def _build_nc():
    """Build the (single, SPMD) Bass graph run on each of the 8 cores."""
    import os
    from contextlib import ExitStack

    import concourse.bass as bass
    from concourse import bacc, bass_isa, mybir, tile
    from concourse.masks import make_identity

    GPS_CH = int(os.environ.get("K_GPS", "24"))    # gpsimd chunks (group 0 only)
    ACT_CH = int(os.environ.get("K_ACT", "40"))    # DVE-mul + ACT-reduce chunks

    F16 = mybir.dt.float16
    F32 = mybir.dt.float32
    AF = mybir.ActivationFunctionType
    OP = mybir.AluOpType
    RED = bass_isa.ReduceOp

    nc = bacc.Bacc(None, target_bir_lowering=False, num_devices=NCORES)

    enc_d = nc.declare_dram_parameter("enc", [BS, L, H], F16, isOutput=False)
    xT_d = nc.declare_dram_parameter("xT", [H, BS], F16, isOutput=False)
    hT_d = nc.declare_dram_parameter("hT", [H, BS], F16, isOutput=False)
    hprev_d = nc.declare_dram_parameter("h_prev", [BS, H], F32, isOutput=False)
    Wi_d = nc.declare_dram_parameter("Wi", [H + 1, 3 * H], F16, isOutput=False)
    Wh_d = nc.declare_dram_parameter("Wh", [H + 1, 3 * H], F16, isOutput=False)
    Wc_d = nc.declare_dram_parameter("Wc", [2 * H + 1, H], F16, isOutput=False)
    Wo_d = nc.declare_dram_parameter("Wo", [H + 1, VSP], F16, isOutput=False)

    out_logits = nc.declare_dram_parameter("out_logits", [B, VSP], F32, isOutput=True)
    out_h = nc.declare_dram_parameter("out_h", [BS, H], F32, isOutput=True)
    out_attn = nc.declare_dram_parameter("out_attn", [BS, L], F32, isOutput=True)

    h16d = nc.dram_tensor("h16d", [BS, H], F16)
    cc_in = nc.dram_tensor("cc_in", [BS, H], F32)
    cc_out = nc.dram_tensor("cc_out", [B, H], F32, addr_space="Shared")

    with tile.TileContext(nc) as tc, ExitStack() as ctx:
        consts = ctx.enter_context(tc.tile_pool(name="consts", bufs=1))
        work = ctx.enter_context(tc.tile_pool(name="work", bufs=2))
        psum = ctx.enter_context(tc.tile_pool(name="psum", bufs=2, space="PSUM"))

        # ---- constants ----
        ident = consts.tile([64, 64], F32)
        make_identity(nc, ident)
        ones8 = consts.tile([1, BS], F16)
        nc.vector.memset(ones8, 1.0)
        ones64 = consts.tile([1, B], F16)
        nc.vector.memset(ones64, 1.0)
        WcK = consts.tile([128, 8, H], F16)
        Wcb = consts.tile([1, H], F16)

        # permanent small state
        co_all = consts.tile([BS, H], F32, tag="co_all")
        h_new = consts.tile([BS, H], F32, tag="h_new")
        h16 = consts.tile([BS, H], F16, tag="h16")
        catT = consts.tile([128, 8, BS], F16)
        hb = consts.tile([128, BS, H], F16)

        # ---- GRU phase (weights in a pool released afterwards) ----
        with tc.tile_pool(name="gru", bufs=1) as grup:
            WiK = grup.tile([128, 4, 3 * H], F16)
            nc.sync.dma_start(
                out=WiK, in_=Wi_d[0:H].rearrange("(c p) n -> p c n", p=128)
            )
            WhK = grup.tile([128, 4, 3 * H], F16)
            nc.sync.dma_start(
                out=WhK, in_=Wh_d[0:H].rearrange("(c p) n -> p c n", p=128)
            )
            Wib = grup.tile([1, 3 * H], F16)
            nc.sync.dma_start(out=Wib, in_=Wi_d[H : H + 1])
            Whbn = grup.tile([1, H], F16)
            nc.sync.dma_start(out=Whbn, in_=Wh_d[H : H + 1, 2 * H : 3 * H])
            xTs = grup.tile([128, 4, BS], F16)
            nc.sync.dma_start(out=xTs, in_=xT_d.rearrange("(c p) m -> p c m", p=128))
            hTs = grup.tile([128, 4, BS], F16)
            nc.sync.dma_start(out=hTs, in_=hT_d.rearrange("(c p) m -> p c m", p=128))
            hpv = grup.tile([BS, H], F32)
            nc.sync.dma_start(out=hpv, in_=hprev_d[:, :])

            # gates r, z: gi+gh accumulated in one PSUM (host folded b_hh's
            # r/z parts into Wi's bias row)
            rz = []
            for g in range(2):
                ps = psum.tile([BS, H], F32, tag="a")
                for c in range(4):
                    nc.tensor.matmul(
                        ps, xTs[:, c, :], WiK[:, c, g * H : (g + 1) * H],
                        start=(c == 0), stop=False,
                    )
                for c in range(4):
                    nc.tensor.matmul(
                        ps, hTs[:, c, :], WhK[:, c, g * H : (g + 1) * H],
                        start=False, stop=False,
                    )
                nc.tensor.matmul(ps, ones8, Wib[:, g * H : (g + 1) * H],
                                 start=False, stop=True)
                act = grup.tile([BS, H], F32, tag=f"gate{g}")
                nc.scalar.activation(act, ps, AF.Sigmoid)
                rz.append(act)
            r_sb, z_sb = rz

            gi_n = psum.tile([BS, H], F32, tag="a")
            for c in range(4):
                nc.tensor.matmul(gi_n, xTs[:, c, :], WiK[:, c, 2 * H : 3 * H],
                                 start=(c == 0), stop=False)
            nc.tensor.matmul(gi_n, ones8, Wib[:, 2 * H : 3 * H],
                             start=False, stop=True)
            gh_n = psum.tile([BS, H], F32, tag="b")
            for c in range(4):
                nc.tensor.matmul(gh_n, hTs[:, c, :], WhK[:, c, 2 * H : 3 * H],
                                 start=(c == 0), stop=False)
            nc.tensor.matmul(gh_n, ones8, Whbn, start=False, stop=True)

            rhn = grup.tile([BS, H], F32, tag="rhn")
            nc.vector.tensor_mul(rhn, r_sb, gh_n)
            tn = grup.tile([BS, H], F32, tag="tn")
            nc.vector.tensor_add(tn, gi_n, rhn)
            n_sb = grup.tile([BS, H], F32, tag="n_sb")
            nc.scalar.activation(n_sb, tn, AF.Tanh)
            dd = grup.tile([BS, H], F32, tag="dd")
            nc.vector.tensor_sub(dd, hpv, n_sb)
            zd = grup.tile([BS, H], F32, tag="zd")
            nc.vector.tensor_mul(zd, z_sb, dd)
            nc.vector.tensor_add(h_new, zd, n_sb)
            nc.sync.dma_start(out=out_h[:, :], in_=h_new)
            nc.scalar.copy(h16, h_new)

        # h_new^T (fp16 for the concat matmul, fp32 for the broadcast)
        for c in range(4):
            tp = psum.tile([128, BS], F32, tag="c")
            nc.tensor.transpose(tp, h_new[:, c * 128 : (c + 1) * 128],
                                ident[0:BS, 0:BS])
            nc.scalar.copy(catT[:, c, :], tp)

        # broadcast each batch row of h_new across all 128 partitions
        # (bounce through a DRAM fp16 scratch; partition-stride-0 DMA reads)
        nc.sync.dma_start(out=h16d[:, :], in_=h16)
        for b in range(BS):
            nc.gpsimd.dma_start(
                out=hb[:, b, :], in_=h16d[b : b + 1, :].to_broadcast([128, H])
            )

        # ---- attention ----
        GS = 8
        attp = ctx.enter_context(tc.tile_pool(name="attp", bufs=1))
        et = attp.tile([128, BS, LC, H], F16)
        for b in range(BS):
            # partition p holds rows l in [16p, 16p+16) -> fully contiguous
            # 16 KiB per-partition DMA reads
            nc.sync.dma_start(
                out=et[:, b],
                in_=enc_d[b].rearrange("(p c) h -> p c h", p=128),
            )
        nc.sync.dma_start(
            out=WcK, in_=Wc_d[0 : 2 * H].rearrange("(c p) n -> p c n", p=128)
        )
        nc.sync.dma_start(out=Wcb, in_=Wc_d[2 * H : 2 * H + 1])
        WoK = attp.tile([128, 4, VSP], F16)
        nc.sync.dma_start(
            out=WoK, in_=Wo_d[0:H].rearrange("(c p) n -> p c n", p=128)
        )
        wobs = []
        for hh in range(2):
            wob = consts.tile([1, VSP // 2], F16, tag=f"wob{hh}")
            nc.sync.dma_start(
                out=wob, in_=Wo_d[H : H + 1, hh * 2048 : (hh + 1) * 2048]
            )
            wobs.append(wob)

        scr = consts.tile([128, BS, LC], F32)
        attn_l = consts.tile([128, BS, LC], F16)
        m1 = consts.tile([128, BS], F32)
        m1n = consts.tile([128, BS], F32)
        z1 = consts.tile([128, BS], F32)
        mg = consts.tile([128, BS], F32)
        zg = consts.tile([128, BS], F32)
        lnz = consts.tile([128, BS], F32)
        negc = consts.tile([128, BS], F32)

        for g in range(1):
            rows = range(g * GS, (g + 1) * GS)
            gsl = slice(g * GS, (g + 1) * GS)
            # -- phase A: score chunks. GPSIMD gets a front-loaded share in
            # group 0 only, so its queue is free for the softmax
            # partition-reduces and the collectives later; the rest is split
            # between DVE fused multiply-accumulate and DVE-mul + ACT-reduce.
            nact = ACT_CH // 2
            chunks = [(b, c) for b in rows for c in range(LC)]
            for j, (b, c) in enumerate(chunks):
                if j < GPS_CH:
                    prod2 = work.tile([128, H], F16, tag="prod2")
                    nc.gpsimd.tensor_mul(prod2, et[:, b, c, :], hb[:, b, :])
                    prod3 = consts.tile([128, H], F16, tag="prod3")
                    nc.scalar.activation(
                        prod3, prod2, AF.Identity,
                        accum_out=scr[:, b, c : c + 1],
                    )
                elif (j % len(chunks)) % 3 == 2 and nact > 0:
                    nact -= 1
                    prod2 = work.tile([128, H], F16, tag="prod2")
                    nc.vector.tensor_mul(prod2, et[:, b, c, :], hb[:, b, :])
                    prod3 = consts.tile([128, H], F16, tag="prod3")
                    nc.scalar.activation(
                        prod3, prod2, AF.Identity,
                        accum_out=scr[:, b, c : c + 1],
                    )
                else:
                    prod = consts.tile([128, H], F16, tag="prod")
                    nc.vector.scalar_tensor_tensor(
                        out=prod, in0=et[:, b, c, :], scalar=1.0,
                        in1=hb[:, b, :],
                        op0=OP.mult, op1=OP.mult,
                        accum_out=scr[:, b, c : c + 1],
                    )

            # -- phase B: softmax batched across the group's rows --
            # per-partition max over l-chunks (one 3D reduce), then global
            # max/sum across partitions; attn = exp(s - Mg - ln Z)
            nc.vector.tensor_reduce(
                m1[:, gsl], scr[:, gsl.start : gsl.stop],
                axis=mybir.AxisListType.X, op=OP.max,
            )
            nc.gpsimd.partition_all_reduce(
                mg[:, gsl], m1[:, gsl], channels=128, reduce_op=RED.max
            )
            nc.vector.tensor_scalar_mul(m1n[:, gsl], mg[:, gsl], -1.0)
            for b in rows:
                ex = work.tile([128, LC], F32, tag="ex")
                nc.scalar.activation(
                    ex, scr[:, b], AF.Exp, bias=m1n[:, b : b + 1],
                    accum_out=z1[:, b : b + 1],
                )
            nc.gpsimd.partition_all_reduce(
                zg[:, gsl], z1[:, gsl], channels=128, reduce_op=RED.add
            )
            nc.scalar.activation(lnz[:, gsl], zg[:, gsl], AF.Ln)
            # negc = -(Mg + lnZ) = -Mg - lnZ
            nc.vector.scalar_tensor_tensor(
                out=negc[:, gsl], in0=lnz[:, gsl], scalar=-1.0, in1=m1n[:, gsl],
                op0=OP.mult, op1=OP.add,
            )

            # -- phase C: attention weights out + context matmuls --
            ctx_g = consts.tile([GS, H], F32, tag="ctx_g")
            for b in rows:
                nc.scalar.activation(
                    attn_l[:, b], scr[:, b], AF.Exp, bias=negc[:, b : b + 1]
                )
                af = work.tile([128, LC], F32, tag="af")
                nc.scalar.activation(af, scr[:, b], AF.Exp,
                                     bias=negc[:, b : b + 1])
                nc.sync.dma_start(
                    out=out_attn[b].rearrange("(p c) -> p c", p=128), in_=af
                )
                ctxp = psum.tile([1, H], F32, tag="b")
                for c in range(LC):
                    nc.tensor.matmul(
                        ctxp, attn_l[:, b, c : c + 1], et[:, b, c, :],
                        start=(c == 0), stop=(c == LC - 1),
                    )
                ctx_b = consts.tile([1, H], F32, tag="ctx_b")
                nc.scalar.copy(ctx_b, ctxp)
                nc.sync.dma_start(out=ctx_g[b - g * GS : b - g * GS + 1, :],
                                  in_=ctx_b)

            # context^T into the concat stationary operand (columns gsl)
            for c in range(4):
                tp2 = psum.tile([128, GS], F32, tag="c")
                nc.tensor.transpose(tp2, ctx_g[:, c * 128 : (c + 1) * 128],
                                    ident[0:GS, 0:GS])
                nc.scalar.copy(catT[:, 4 + c, gsl], tp2)

            # -- concat -> linear -> tanh for this group's rows --
            gcp = psum.tile([GS, H], F32, tag="c")
            for j in range(8):
                nc.tensor.matmul(gcp, catT[:, j, gsl], WcK[:, j, :],
                                 start=(j == 0), stop=False)
            nc.tensor.matmul(gcp, ones8[:, 0:GS], Wcb, start=False, stop=True)
            co_g = consts.tile([GS, H], F32, tag="co_g")
            nc.scalar.activation(co_g, gcp, AF.Tanh)
            nc.sync.dma_start(
                out=co_all[g * GS : (g + 1) * GS, :], in_=co_g
            )

            if True:
                # -- all-gather the concat activations (one collective),
                # then the vocab-sharded projection over all 64 rows --
                nc.sync.dma_start(out=cc_in[:, :], in_=co_all)
                nc.gpsimd.collective_compute(
                    "AllGather",
                    OP.bypass,
                    replica_groups=[list(range(NCORES))],
                    ins=[cc_in.ap().opt()],
                    outs=[cc_out.ap().opt()],
                )
                ca = consts.tile([B, H], F32, tag="ca")
                nc.sync.dma_start(out=ca, in_=cc_out[:, :])
                caT = consts.tile([128, 4, B], F16, tag="caT")
                for c in range(4):
                    tp3 = psum.tile([128, B], F32, tag="c")
                    nc.tensor.transpose(tp3, ca[:, c * 128 : (c + 1) * 128],
                                        ident[0:B, 0:B])
                    nc.scalar.copy(caT[:, c, :], tp3)

                for nch in range(8):
                    lp = psum.tile([B, 512], F32, tag="a")
                    for c in range(4):
                        nc.tensor.matmul(
                            lp, caT[:, c, :],
                            WoK[:, c, nch * 512 : (nch + 1) * 512],
                            start=(c == 0), stop=False,
                        )
                    nc.tensor.matmul(
                        lp, ones64,
                        wobs[nch // 4][:, (nch % 4) * 512 : (nch % 4 + 1) * 512],
                        start=False, stop=True,
                    )
                    lsb = work.tile([B, 512], F16, tag="lsb")
                    nc.scalar.copy(lsb, lp)
                    # SWDGE cast back to f32 on the way out
                    nc.gpsimd.dma_start(
                        out=out_logits[:, nch * 512 : (nch + 1) * 512],
                        in_=lsb,
                    )

    nc.compile()
    return nc


def make_in_maps(inputs):
    """Shard + cast the full inputs into per-core input maps."""
    f16 = np.float16
    f32 = np.float32
    emb = np.asarray(inputs["emb_table"], f32)
    seq = np.asarray(inputs["input_seq"]).astype(np.int64)
    h0 = np.asarray(inputs["last_hidden"], f32)[0]          # (64, 512)
    enc = np.asarray(inputs["encoder_outputs"], f32)        # (64, 2048, 512)
    W_ih = np.asarray(inputs["W_ih"], f32)
    W_hh = np.asarray(inputs["W_hh"], f32)
    b_ih = np.asarray(inputs["b_ih"], f32)
    b_hh = np.asarray(inputs["b_hh"], f32)
    W_concat = np.asarray(inputs["W_concat"], f32)
    b_concat = np.asarray(inputs["b_concat"], f32)
    W_out = np.asarray(inputs["W_out"], f32)
    b_out = np.asarray(inputs["b_out"], f32)

    x = emb[seq]                                            # (64, 512)
    enc16 = enc.astype(f16)
    # fold b_hh's r/z parts into Wi's bias row (kernel adds Wi's bias once
    # for r/z); Wh's bias row only contributes its n part
    bi = b_ih.copy()
    bi[0 : 2 * H] += b_hh[0 : 2 * H]
    Wi = np.ascontiguousarray(np.vstack([W_ih.T, bi[None]]).astype(f16))
    Wh = np.ascontiguousarray(np.vstack([W_hh.T, b_hh[None]]).astype(f16))
    Wc = np.ascontiguousarray(np.vstack([W_concat.T, b_concat[None]]).astype(f16))

    in_maps = []
    for i in range(NCORES):
        bs = slice(i * BS, (i + 1) * BS)
        vs = slice(i * VS, (i + 1) * VS)
        Wo = np.zeros((H + 1, VSP), f16)
        Wo[0:H, 0:VS] = W_out[vs].T
        Wo[H, 0:VS] = b_out[vs]
        in_maps.append(
            {
                "enc": np.ascontiguousarray(enc16[bs]),
                "xT": np.ascontiguousarray(x[bs].T.astype(f16)),
                "hT": np.ascontiguousarray(h0[bs].T.astype(f16)),
                "h_prev": np.ascontiguousarray(h0[bs]),
                "Wi": Wi,
                "Wh": Wh,
                "Wc": Wc,
                "Wo": Wo,
            }
        )
    return in_maps


def assemble_outputs(results, inputs):
    f32 = np.float32
    output = np.concatenate(
        [np.asarray(results[i]["out_logits"], f32)[:, :VS] for i in range(NCORES)],
        axis=1,
    )
    h_new = np.concatenate(
        [np.asarray(results[i]["out_h"], f32) for i in range(NCORES)], axis=0
    )[None]
    attn = np.concatenate(
        [np.asarray(results[i]["out_attn"], f32).reshape(BS, L) for i in range(NCORES)],
        axis=0,
    )[:, None, :]
    feed = np.asarray(inputs["input_feeding_prev_time_step"], f32)
    return output, h_new, attn, feed


def get_nc():
    if "nc" not in _CACHE:
        _CACHE["nc"] = _build_nc()
    return _CACHE["nc"]


def kernel(**inputs):
    from concourse.bass_utils import run_bass_kernel_spmd

    nc = get_nc()
    in_maps = make_in_maps(inputs)
    res = run_bass_kernel_spmd(nc, in_maps, core_ids=list(range(NCORES))).results
    return assemble_outputs(res, inputs)
